# revision 29
# baseline (speedup 1.0000x reference)
"""Multi-query attention (nn_Attention) Trainium2 Bass kernel, 8-core SPMD.

Reference computation (fp32):
    q = einsum('bnd,hde->bhne', x, Wq) * dh**-0.5
    k, v = split(x @ Wkv)                      # shared across heads (MQA)
    out = softmax(q @ k^T) @ v                 # per head
    out = concat_heads(out) @ Wout

Shapes: x [2,2048,1024], Wq [16,1024,64], Wkv [1024,128], Wout [1024,1024].

Sharding: core = b*4 + g handles batch b and heads [4g, 4g+4). Wout is split
along its input (inner) dim, so each core produces a partial [2048,1024]
output; the host sums the 4 partials per batch.

Per-core pipeline (matmuls in fp32r = full-rate single-pass fp32):
  1. xT[d,n] built from x via PE transposes (exact data movement).
  2. qT (heads packed in pairs), kT (pre-duplicated to both 64-partition
     halves via a [Wk|Wk] packed weight), vT; v[n,e] recovered from vT by PE
     transpose and augmented with a ones column (fused softmax denominator).
  3. For each head pair and 512-wide i-tile: simT[j,i] tiles via two
     concurrent row-tiled (K=64) matmuls into one 2-bank PSUM tile;
     one ACT exp over both (scale folded in; no max subtraction needed -
     scores are O(1)); two accumulating matmuls with lhsT=v_aug give
     oT[e,i] plus the row-sums in PSUM partition 64.
  4. Normalize: oT accumulates in two 8-step PSUM halves (recycles
     accumulator banks early); halves are summed on DVE, the sums row is
     broadcast with a K=1 matmul, approx-reciprocal'd on DVE, and
     multiplied in. The normalize tails and output-projection groups are
     deferred and drip-fed into later j-loops to keep all engines busy.
  5. Output projection from oTn against the core's Wout rows; DMA out.
"""

import os

import numpy as np

import concourse.mybir as mybir
import concourse.tile as tile
from concourse import bacc
from concourse.bass_utils import run_bass_kernel_spmd
from concourse.dve_ops import RECIP_APPROX_FAST_CONSTS, RECIPROCAL_APPROX_FAST
from concourse.masks import make_identity

DIM = 1024
DIM_HEAD = 64
HEADS = 16
SCALE = DIM_HEAD**-0.5
B = 2
N = 2048
N_CORES = 8
HEADS_PER_CORE = HEADS // 4  # 4 head-groups across cores

P = 128
KT = DIM // P  # 8 contraction tiles
NT = N // P  # 16 row tiles of 128
IT = N // 512  # 4 i-tiles of 512
PAIRS = HEADS_PER_CORE // 2  # 2 head pairs
INNER = HEADS_PER_CORE * DIM_HEAD  # 256 per-core inner dim
CHUNKS = INNER // P  # 2 chunks of the inner dim
WKV_COLS = 192  # [Wk | Wk | Wv]


def _build():
    f32 = mybir.dt.float32
    f32r = mybir.dt.float32r
    Exp = mybir.ActivationFunctionType.Exp

    nc = bacc.Bacc("TRN2", target_bir_lowering=False, debug=False,
                   enable_asserts=False)

    x_d = nc.dram_tensor("x", [N, DIM], f32, kind="ExternalInput")
    wq_d = nc.dram_tensor("wq", [DIM, INNER], f32r, kind="ExternalInput")
    wkv_d = nc.dram_tensor("wkv", [DIM, WKV_COLS], f32r, kind="ExternalInput")
    wout_d = nc.dram_tensor("wout", [INNER, DIM], f32r, kind="ExternalInput")
    out_d = nc.dram_tensor("out", [N, DIM], f32, kind="ExternalOutput")

    with tile.TileContext(nc) as tc:
        with (
            tc.tile_pool(name="const", bufs=1) as const,
            tc.tile_pool(name="w", bufs=1) as w,
            tc.tile_pool(name="big", bufs=1) as big,
            tc.tile_pool(name="stage", bufs=4) as stage,
            tc.tile_pool(name="expp", bufs=4) as expp,
            tc.tile_pool(name="small", bufs=2) as small,
            tc.tile_pool(name="outp", bufs=3) as outp,
            tc.tile_pool(name="ps_small", bufs=2, space="PSUM") as ps_small,
            tc.tile_pool(name="ps_sim", bufs=2, space="PSUM") as ps_sim,
            tc.tile_pool(name="ps_acc", bufs=2, space="PSUM") as ps_acc,
        ):
            identity = const.tile([P, P], f32)
            make_identity(nc, identity[:])

            # x tiles for group 0 first, then the weights the first
            # projections need, then the remaining x tiles.
            xss = [
                stage.tile([P, DIM], f32, tag="xs", name=f"xs{it}")
                for it in range(NT)
            ]
            for it in range(4):
                nc.sync.dma_start(xss[it][:], x_d[it * P:(it + 1) * P, :])
            wkv_sb = w.tile([P, KT, WKV_COLS], f32r)
            nc.sync.dma_start(wkv_sb[:], wkv_d[:].rearrange("(ko p) m -> p ko m", p=P))
            wq_sb = w.tile([P, KT, INNER], f32r)
            nc.sync.dma_start(wq_sb[:], wq_d[:].rearrange("(ko p) m -> p ko m", p=P))
            for it in range(4, NT):
                nc.sync.dma_start(xss[it][:], x_d[it * P:(it + 1) * P, :])
            wout_sb = w.tile([P, CHUNKS, DIM], f32r)
            nc.sync.dma_start(
                wout_sb[:], wout_d[:].rearrange("(c p) m -> p c m", p=P)
            )

            onescol = const.tile([P, 1], f32)
            nc.gpsimd.memset(onescol[:], 1.0)
            ones65f = const.tile([65, 64], f32)
            nc.gpsimd.memset(ones65f[:], 1.0)
            ones65 = const.tile([65, 64], f32r)
            nc.vector.tensor_copy(ones65[64:65, :], ones65f[64:65, :])

            xT = big.tile([P, KT, N], f32r)
            kT2 = big.tile([P, N], f32r)  # [kT; kT] stacked halves
            vT = big.tile([64, N], f32)
            v_aug = big.tile([P, NT, 65], f32r)
            nc.vector.tensor_copy(
                v_aug[:, :, 64:65], onescol[:, None, :].to_broadcast((P, NT, 1))
            )
            qT = big.tile([P, PAIRS, N], f32r)
            oTn = big.tile([P, CHUNKS, N], f32r)
            rc = RECIP_APPROX_FAST_CONSTS

            def qt_proj(p, it):
                isl = slice(it * 512, (it + 1) * 512)
                psq = ps_small.tile([P, 512], f32, tag="pss")
                for kt in range(KT):
                    nc.tensor.matmul(
                        psq[:],
                        wq_sb[:, kt, p * P:(p + 1) * P],
                        xT[:, kt, isl],
                        start=(kt == 0),
                        stop=(kt == KT - 1),
                    )
                nc.vector.tensor_copy(qT[:, p, isl], psq[:])

            def transpose_unit(it):
                pst = ps_sim.tile([P, KT, P], f32, tag="sim")
                for s in range(KT):
                    nc.tensor.matmul(
                        pst[:, s, :],
                        xss[it][:, s * P:(s + 1) * P],
                        identity[:],
                        is_transpose=True,
                        start=(s % 4 == 0),
                        stop=(s % 4 == 3),
                    )
                nc.scalar.copy(xT[:, :, it * P:(it + 1) * P], pst[:])

            def kv_unit(g):
                isl = slice(g * 512, (g + 1) * 512)
                psk = ps_small.tile([P, 512], f32, tag="pss")
                for kt in range(KT):
                    nc.tensor.matmul(
                        psk[:],
                        wkv_sb[:, kt, 0:P],
                        xT[:, kt, isl],
                        start=(kt == 0),
                        stop=(kt == KT - 1),
                    )
                nc.scalar.copy(kT2[:, isl], psk[:])

            def v_unit(g):
                isl = slice(g * 512, (g + 1) * 512)
                psv = ps_small.tile([64, 512], f32, tag="pss")
                for kt in range(KT):
                    nc.tensor.matmul(
                        psv[:],
                        wkv_sb[:, kt, P:P + 64],
                        xT[:, kt, isl],
                        start=(kt == 0),
                        stop=(kt == KT - 1),
                    )
                nc.vector.tensor_copy(vT[:, isl], psv[:])
                psvt = ps_small.tile([P, 4, 64], f32, tag="pss")
                for s in range(4):
                    jt = g * 4 + s
                    nc.tensor.matmul(
                        psvt[:, s, :],
                        vT[:, jt * P:(jt + 1) * P],
                        identity[0:64, 0:64],
                        is_transpose=True,
                        start=(s == 0),
                        stop=(s == 3),
                    )
                nc.vector.tensor_copy(
                    v_aug[:, g * 4:(g + 1) * 4, 0:64], psvt[:]
                )

            def prologue_group(g):
                for t in range(4):
                    transpose_unit(g * 4 + t)
                kv_unit(g)
                v_unit(g)

            pending = []  # deferred outproj groups

            def emit_jt(it, p, po, jt):
                isl = slice(it * 512, (it + 1) * 512)
                jsl = slice(jt * P, (jt + 1) * P)
                pss = ps_sim.tile([P, 2, 512], f32, tag="sim")
                for h in range(2):
                    nc.tensor.matmul(
                        pss[:, h, :],
                        kT2[64 * h:64 * (h + 1), jsl],
                        qT[64 * h:64 * (h + 1), p, isl],
                        tile_position=(64 * h, 0),
                    )
                et = expp.tile([P, 2, 512], f32r, tag="exp")
                nc.scalar.activation(et[:], pss[:], Exp, scale=SCALE)
                for h in range(2):
                    nc.tensor.matmul(
                        po[h][:],
                        v_aug[:, jt, :],
                        et[:, h, :],
                        start=(jt % 8 == 0),
                        stop=(jt % 8 == 7),
                    )
                if jt % 2 == 1 and pending:
                    pending.pop(0)()

            def alloc_po(it, p, half):
                return [
                    ps_acc.tile(
                        [65, 512], f32, tag="po", name=f"po{h}_{p}_{it}_{half}"
                    )
                    for h in range(2)
                ]

            def flush_half(po, oh):
                """Copy the first-half oT accumulation to SBUF, freeing PSUM."""
                for h in range(2):
                    nc.vector.tensor_copy(oh[h][:], po[h][:])

            def alloc_oh(it, p):
                return [
                    small.tile([65, 512], f32r, tag=f"oh{h}", name=f"oh{h}_{p}_{it}")
                    for h in range(2)
                ]

            def norm_tail(it, p, h, ou):
                isl = slice(it * 512, (it + 1) * 512)
                psb = ps_small.tile([64, 512], f32, tag="pss")
                nc.tensor.matmul(psb[:], ones65[64:65, :], ou[64:65, :])
                rbc = small.tile([64, 512], f32, tag="rbc")
                nc.vector._custom_dve(
                    RECIPROCAL_APPROX_FAST,
                    out=rbc[:],
                    in0=psb[:],
                    s0=rc["s0"],
                    s1=rc["s1"],
                    imm2=rc["imm2"],
                )
                nc.vector.tensor_tensor(
                    oTn[64 * h:64 * (h + 1), p, isl],
                    ou[0:64, :],
                    rbc[:],
                    mybir.AluOpType.mult,
                )

            def emit_normalize(it, p, oh, po2, defer=False):
                for h in range(2):
                    ou = small.tile([65, 512], f32r, tag=f"ou{h}")
                    nc.vector.tensor_tensor(
                        ou[:], oh[h][:], po2[h][:], mybir.AluOpType.add
                    )
                    if defer:
                        pending.append(
                            lambda it=it, p=p, h=h, ou=ou: norm_tail(it, p, h, ou)
                        )
                    else:
                        norm_tail(it, p, h, ou)

            def outproj_group(itt, dh, pool=None):
                dsl = slice(dh * 512, (dh + 1) * 512)
                if pool is None:
                    pso = ps_small.tile([P, 512], f32, tag="pss")
                else:
                    pso = pool.tile([P, KT, P], f32, tag="sim", name=f"pso_{itt}_{dh}")[:, 0:4, :]
                for c in range(CHUNKS):
                    nc.tensor.matmul(
                        pso.opt(),
                        oTn[:, c, itt * P:(itt + 1) * P],
                        wout_sb[:, c, dsl],
                        start=(c == 0),
                        stop=(c == CHUNKS - 1),
                    )
                os_ = outp.tile([P, 512], f32, tag="os")
                if pool is not None:
                    nc.scalar.copy(os_[:], pso.opt())
                else:
                    nc.vector.tensor_copy(os_[:], pso.opt())
                nc.sync.dma_start(out_d[itt * P:(itt + 1) * P, dsl], os_[:])

            def queue_outproj(it):
                last = it == IT - 1
                for t in range(4):
                    for dh in range(2):
                        pool = ps_sim if last and (t + dh) % 2 else None
                        pending.append(
                            lambda itt=it * 4 + t, d=dh, pl=pool: outproj_group(
                                itt, d, pl
                            )
                        )

            # ---- Prologue woven with the it=0 j-loops, unit-interleaved ----
            def weave(units, jts):
                q = list(jts)
                for i, u in enumerate(units):
                    u()
                    if i < len(units) - 1 and q:
                        it_, p_, po_, jt_ = q.pop(0)
                        emit_jt(it_, p_, po_, jt_)
                for it_, p_, po_, jt_ in q:
                    emit_jt(it_, p_, po_, jt_)

            prologue_group(0)
            qt_proj(0, 0)
            qt_proj(1, 0)
            a0 = alloc_po(0, 0, 0)
            weave(
                [lambda t=t: transpose_unit(4 + t) for t in range(4)]
                + [lambda: kv_unit(1), lambda: v_unit(1)],
                [(0, 0, a0, jt) for jt in range(0, 4)],
            )
            weave(
                [lambda t=t: transpose_unit(8 + t) for t in range(4)]
                + [lambda: kv_unit(2), lambda: v_unit(2)],
                [(0, 0, a0, jt) for jt in range(4, 8)],
            )
            ah = alloc_oh(0, 0)
            flush_half(a0, ah)
            qt_proj(0, 1)
            b0 = alloc_po(0, 1, 0)
            weave(
                [lambda t=t: transpose_unit(12 + t) for t in range(4)]
                + [lambda: kv_unit(3), lambda: v_unit(3)],
                [(0, 1, b0, jt) for jt in range(0, 4)],
            )
            for jt in range(4, 8):
                emit_jt(0, 1, b0, jt)
            bh = alloc_oh(0, 1)
            flush_half(b0, bh)
            qt_proj(1, 1)
            a1 = alloc_po(0, 0, 1)
            for jt in range(8, 16):
                emit_jt(0, 0, a1, jt)
            emit_normalize(0, 0, ah, a1, defer=True)
            b1 = alloc_po(0, 1, 1)
            for jt in range(8, 16):
                emit_jt(0, 1, b1, jt)
            emit_normalize(0, 1, bh, b1, defer=True)
            queue_outproj(0)

            # ---- Remaining (it, p) j-loops ----
            seq = [(it, p) for it in range(1, IT) for p in range(PAIRS)]
            for it, p in seq:
                po = alloc_po(it, p, 0)
                for jt in range(0, 8):
                    emit_jt(it, p, po, jt)
                oh = alloc_oh(it, p)
                flush_half(po, oh)
                po2 = alloc_po(it, p, 1)
                for jt in range(8, 16):
                    emit_jt(it, p, po2, jt)
                    if jt == 12 and it + 1 < IT:
                        qt_proj(p, it + 1)
                emit_normalize(it, p, oh, po2, defer=(it, p) != (IT - 1, PAIRS - 1))
                if p == PAIRS - 1:
                    queue_outproj(it)
            for fn in pending:
                fn()
            del pending[:]

    nc.compile()
    return nc


_NC = None


def _get_nc():
    global _NC
    if _NC is None:
        _NC = _build()
    return _NC


def _prep_in_maps(x, Wq, Wkv, Wout):
    in_maps = []
    wk = Wkv[:, 0:DIM_HEAD]
    wv = Wkv[:, DIM_HEAD:]
    wkv_packed = np.ascontiguousarray(
        np.concatenate([wk, wk, wv], axis=1), dtype=np.float32
    )
    for core in range(N_CORES):
        b, g = divmod(core, 4)
        h0 = g * HEADS_PER_CORE
        wq = np.ascontiguousarray(
            np.transpose(Wq[h0:h0 + HEADS_PER_CORE], (1, 0, 2)).reshape(DIM, INNER),
            dtype=np.float32,
        )
        wout = np.ascontiguousarray(
            Wout[h0 * DIM_HEAD:(h0 + HEADS_PER_CORE) * DIM_HEAD], dtype=np.float32
        )
        in_maps.append(
            {
                "x": np.ascontiguousarray(x[b], dtype=np.float32),
                "wq": wq,
                "wkv": wkv_packed,
                "wout": wout,
            }
        )
    return in_maps


def _ensure_hook_shim():
    """bass_utils imports antenv.axon_hooks when tracing is requested via
    env (BASS_TRACE); that module is absent on this image. Provide a no-op
    fallback so an inherited env var cannot break a plain run."""
    try:
        import antenv.axon_hooks  # noqa: F401
    except Exception:
        import sys
        import types

        m = types.ModuleType("antenv.axon_hooks")
        m.get_axon_ntff_profile_hook = lambda: None
        m.set_axon_ntff_profile_hook = lambda h: None
        sys.modules["antenv.axon_hooks"] = m


def run(inputs, trace=False):
    """Run on 8 cores; returns (full_output, BassKernelResults)."""
    _ensure_hook_shim()
    nc = _get_nc()
    in_maps = _prep_in_maps(
        np.asarray(inputs["x"]),
        np.asarray(inputs["Wq"]),
        np.asarray(inputs["Wkv"]),
        np.asarray(inputs["Wout"]),
    )
    res = run_bass_kernel_spmd(
        nc, in_maps, core_ids=list(range(N_CORES)), trace=trace
    )
    out = np.zeros((B, N, DIM), dtype=np.float32)
    for core in range(N_CORES):
        b = core // 4
        out[b] += res.results[core]["out"]
    return out, res


def kernel(**inputs) -> np.ndarray:
    out, _ = run(inputs, trace=bool(os.environ.get("BASS_KERNEL_TRACE")))
    return out


# revision 30
# speedup vs baseline: 1.0014x; 1.0014x over previous
"""Multi-query attention (nn_Attention) Trainium2 Bass kernel, 8-core SPMD.

Reference computation (fp32):
    q = einsum('bnd,hde->bhne', x, Wq) * dh**-0.5
    k, v = split(x @ Wkv)                      # shared across heads (MQA)
    out = softmax(q @ k^T) @ v                 # per head
    out = concat_heads(out) @ Wout

Shapes: x [2,2048,1024], Wq [16,1024,64], Wkv [1024,128], Wout [1024,1024].

Sharding: core = b*4 + g handles batch b and heads [4g, 4g+4). Wout is split
along its input (inner) dim, so each core produces a partial [2048,1024]
output; the host sums the 4 partials per batch.

Per-core pipeline (matmuls in fp32r = full-rate single-pass fp32):
  1. xT[d,n] built from x via PE transposes (exact data movement).
  2. qT (heads packed in pairs), kT (pre-duplicated to both 64-partition
     halves via a [Wk|Wk] packed weight), vT; v[n,e] recovered from vT by PE
     transpose and augmented with a ones column (fused softmax denominator).
  3. For each head pair and 512-wide i-tile: simT[j,i] tiles via two
     concurrent row-tiled (K=64) matmuls into one 2-bank PSUM tile;
     one ACT exp over both (scale folded in; no max subtraction needed -
     scores are O(1)); two accumulating matmuls with lhsT=v_aug give
     oT[e,i] plus the row-sums in PSUM partition 64.
  4. Normalize: oT accumulates in two 8-step PSUM halves (recycles
     accumulator banks early); halves are summed on DVE, the sums row is
     broadcast with a K=1 matmul, approx-reciprocal'd on DVE, and
     multiplied in. The normalize tails and output-projection groups are
     deferred and drip-fed into later j-loops to keep all engines busy.
  5. Output projection from oTn against the core's Wout rows; DMA out.
"""

import os

import numpy as np

import concourse.mybir as mybir
import concourse.tile as tile
from concourse import bacc
from concourse.bass_utils import run_bass_kernel_spmd
from concourse.dve_ops import RECIP_APPROX_FAST_CONSTS, RECIPROCAL_APPROX_FAST
from concourse.masks import make_identity

DIM = 1024
DIM_HEAD = 64
HEADS = 16
SCALE = DIM_HEAD**-0.5
B = 2
N = 2048
N_CORES = 8
HEADS_PER_CORE = HEADS // 4  # 4 head-groups across cores

P = 128
KT = DIM // P  # 8 contraction tiles
NT = N // P  # 16 row tiles of 128
IT = N // 512  # 4 i-tiles of 512
PAIRS = HEADS_PER_CORE // 2  # 2 head pairs
INNER = HEADS_PER_CORE * DIM_HEAD  # 256 per-core inner dim
CHUNKS = INNER // P  # 2 chunks of the inner dim
WKV_COLS = 192  # [Wk | Wk | Wv]


def _build():
    f32 = mybir.dt.float32
    f32r = mybir.dt.float32r
    Exp = mybir.ActivationFunctionType.Exp

    nc = bacc.Bacc("TRN2", target_bir_lowering=False, debug=False,
                   enable_asserts=False)

    x_d = nc.dram_tensor("x", [N, DIM], f32, kind="ExternalInput")
    wq_d = nc.dram_tensor("wq", [DIM, INNER], f32r, kind="ExternalInput")
    wkv_d = nc.dram_tensor("wkv", [DIM, WKV_COLS], f32r, kind="ExternalInput")
    wout_d = nc.dram_tensor("wout", [INNER, DIM], f32r, kind="ExternalInput")
    out_d = nc.dram_tensor("out", [N, DIM], f32, kind="ExternalOutput")

    with tile.TileContext(nc) as tc:
        with (
            tc.tile_pool(name="const", bufs=1) as const,
            tc.tile_pool(name="w", bufs=1) as w,
            tc.tile_pool(name="big", bufs=1) as big,
            tc.tile_pool(name="stage", bufs=4) as stage,
            tc.tile_pool(name="expp", bufs=4) as expp,
            tc.tile_pool(name="small", bufs=2) as small,
            tc.tile_pool(name="outp", bufs=3) as outp,
            tc.tile_pool(name="ps_small", bufs=2, space="PSUM") as ps_small,
            tc.tile_pool(name="ps_sim", bufs=2, space="PSUM") as ps_sim,
            tc.tile_pool(name="ps_acc", bufs=2, space="PSUM") as ps_acc,
        ):
            identity = const.tile([P, P], f32)
            make_identity(nc, identity[:])

            # x tiles for group 0 first, then the weights the first
            # projections need, then the remaining x tiles.
            xss = [
                stage.tile([P, DIM], f32, tag="xs", name=f"xs{it}")
                for it in range(NT)
            ]
            for it in range(4):
                nc.sync.dma_start(xss[it][:], x_d[it * P:(it + 1) * P, :])
            wkv_sb = w.tile([P, KT, WKV_COLS], f32r)
            nc.sync.dma_start(wkv_sb[:], wkv_d[:].rearrange("(ko p) m -> p ko m", p=P))
            wq_sb = w.tile([P, KT, INNER], f32r)
            nc.sync.dma_start(wq_sb[:], wq_d[:].rearrange("(ko p) m -> p ko m", p=P))
            for it in range(4, NT):
                nc.sync.dma_start(xss[it][:], x_d[it * P:(it + 1) * P, :])
            wout_sb = w.tile([P, CHUNKS, DIM], f32r)
            nc.sync.dma_start(
                wout_sb[:], wout_d[:].rearrange("(c p) m -> p c m", p=P)
            )

            onescol = const.tile([P, 1], f32)
            nc.gpsimd.memset(onescol[:], 1.0)
            ones65f = const.tile([65, 64], f32)
            nc.gpsimd.memset(ones65f[:], 1.0)
            ones65 = const.tile([65, 64], f32r)
            nc.vector.tensor_copy(ones65[64:65, :], ones65f[64:65, :])

            xT = big.tile([P, KT, N], f32r)
            kT2 = big.tile([P, N], f32r)  # [kT; kT] stacked halves
            vT = big.tile([64, N], f32)
            v_aug = big.tile([P, NT, 65], f32r)
            nc.vector.tensor_copy(
                v_aug[:, :, 64:65], onescol[:, None, :].to_broadcast((P, NT, 1))
            )
            qT = big.tile([P, PAIRS, N], f32r)
            oTn = big.tile([P, CHUNKS, N], f32r)
            rc = RECIP_APPROX_FAST_CONSTS

            def qt_proj(p, it):
                isl = slice(it * 512, (it + 1) * 512)
                psq = ps_small.tile([P, 512], f32, tag="pss")
                for kt in range(KT):
                    nc.tensor.matmul(
                        psq[:],
                        wq_sb[:, kt, p * P:(p + 1) * P],
                        xT[:, kt, isl],
                        start=(kt == 0),
                        stop=(kt == KT - 1),
                    )
                nc.vector.tensor_copy(qT[:, p, isl], psq[:])

            def transpose_unit(it):
                pst = ps_sim.tile([P, KT, P], f32, tag="sim")
                for s in range(KT):
                    nc.tensor.matmul(
                        pst[:, s, :],
                        xss[it][:, s * P:(s + 1) * P],
                        identity[:],
                        is_transpose=True,
                        start=(s % 4 == 0),
                        stop=(s % 4 == 3),
                    )
                nc.scalar.copy(xT[:, :, it * P:(it + 1) * P], pst[:])

            def kv_unit(g):
                isl = slice(g * 512, (g + 1) * 512)
                psk = ps_small.tile([P, 512], f32, tag="pss")
                for kt in range(KT):
                    nc.tensor.matmul(
                        psk[:],
                        wkv_sb[:, kt, 0:P],
                        xT[:, kt, isl],
                        start=(kt == 0),
                        stop=(kt == KT - 1),
                    )
                nc.scalar.copy(kT2[:, isl], psk[:])

            def v_unit(g):
                isl = slice(g * 512, (g + 1) * 512)
                psv = ps_small.tile([64, 512], f32, tag="pss")
                for kt in range(KT):
                    nc.tensor.matmul(
                        psv[:],
                        wkv_sb[:, kt, P:P + 64],
                        xT[:, kt, isl],
                        start=(kt == 0),
                        stop=(kt == KT - 1),
                    )
                nc.vector.tensor_copy(vT[:, isl], psv[:])
                psvt = ps_small.tile([P, 4, 64], f32, tag="pss")
                for s in range(4):
                    jt = g * 4 + s
                    nc.tensor.matmul(
                        psvt[:, s, :],
                        vT[:, jt * P:(jt + 1) * P],
                        identity[0:64, 0:64],
                        is_transpose=True,
                        start=(s == 0),
                        stop=(s == 3),
                    )
                nc.vector.tensor_copy(
                    v_aug[:, g * 4:(g + 1) * 4, 0:64], psvt[:]
                )

            def prologue_group(g):
                for t in range(4):
                    transpose_unit(g * 4 + t)
                kv_unit(g)
                v_unit(g)

            pending = []  # deferred outproj groups

            skewq = []

            def flush_skew():
                while skewq:
                    skewq.pop(0)()

            def emit_jt(it, p, po, jt):
                isl = slice(it * 512, (it + 1) * 512)
                jsl = slice(jt * P, (jt + 1) * P)
                pss = ps_sim.tile([P, 2, 512], f32, tag="sim")
                for h in range(2):
                    nc.tensor.matmul(
                        pss[:, h, :],
                        kT2[64 * h:64 * (h + 1), jsl],
                        qT[64 * h:64 * (h + 1), p, isl],
                        tile_position=(64 * h, 0),
                    )
                et = expp.tile([P, 2, 512], f32r, tag="exp")
                nc.scalar.activation(et[:], pss[:], Exp, scale=SCALE)
                flush_skew()

                def do_oT(po=po, jt=jt, et=et):
                    for h in range(2):
                        nc.tensor.matmul(
                            po[h][:],
                            v_aug[:, jt, :],
                            et[:, h, :],
                            start=(jt % 8 == 0),
                            stop=(jt % 8 == 7),
                        )

                skewq.append(do_oT)
                if jt % 2 == 1 and pending:
                    pending.pop(0)()

            def alloc_po(it, p, half):
                return [
                    ps_acc.tile(
                        [65, 512], f32, tag="po", name=f"po{h}_{p}_{it}_{half}"
                    )
                    for h in range(2)
                ]

            def flush_half(po, oh):
                """Copy the first-half oT accumulation to SBUF, freeing PSUM."""
                flush_skew()
                for h in range(2):
                    nc.vector.tensor_copy(oh[h][:], po[h][:])

            def alloc_oh(it, p):
                return [
                    small.tile([65, 512], f32r, tag=f"oh{h}", name=f"oh{h}_{p}_{it}")
                    for h in range(2)
                ]

            def norm_tail(it, p, h, ou):
                isl = slice(it * 512, (it + 1) * 512)
                psb = ps_small.tile([64, 512], f32, tag="pss")
                nc.tensor.matmul(psb[:], ones65[64:65, :], ou[64:65, :])
                rbc = small.tile([64, 512], f32, tag="rbc")
                nc.vector._custom_dve(
                    RECIPROCAL_APPROX_FAST,
                    out=rbc[:],
                    in0=psb[:],
                    s0=rc["s0"],
                    s1=rc["s1"],
                    imm2=rc["imm2"],
                )
                nc.vector.tensor_tensor(
                    oTn[64 * h:64 * (h + 1), p, isl],
                    ou[0:64, :],
                    rbc[:],
                    mybir.AluOpType.mult,
                )

            def emit_normalize(it, p, oh, po2, defer=False):
                flush_skew()
                for h in range(2):
                    ou = small.tile([65, 512], f32r, tag=f"ou{h}")
                    nc.vector.tensor_tensor(
                        ou[:], oh[h][:], po2[h][:], mybir.AluOpType.add
                    )
                    if defer:
                        pending.append(
                            lambda it=it, p=p, h=h, ou=ou: norm_tail(it, p, h, ou)
                        )
                    else:
                        norm_tail(it, p, h, ou)

            def outproj_group(itt, dh, pool=None):
                dsl = slice(dh * 512, (dh + 1) * 512)
                if pool is None:
                    pso = ps_small.tile([P, 512], f32, tag="pss")
                else:
                    pso = pool.tile([P, KT, P], f32, tag="sim", name=f"pso_{itt}_{dh}")[:, 0:4, :]
                for c in range(CHUNKS):
                    nc.tensor.matmul(
                        pso.opt(),
                        oTn[:, c, itt * P:(itt + 1) * P],
                        wout_sb[:, c, dsl],
                        start=(c == 0),
                        stop=(c == CHUNKS - 1),
                    )
                os_ = outp.tile([P, 512], f32, tag="os")
                if pool is not None:
                    nc.scalar.copy(os_[:], pso.opt())
                else:
                    nc.vector.tensor_copy(os_[:], pso.opt())
                nc.sync.dma_start(out_d[itt * P:(itt + 1) * P, dsl], os_[:])

            def queue_outproj(it):
                last = it == IT - 1
                for t in range(4):
                    for dh in range(2):
                        pool = ps_sim if last and (t + dh) % 2 else None
                        pending.append(
                            lambda itt=it * 4 + t, d=dh, pl=pool: outproj_group(
                                itt, d, pl
                            )
                        )

            # ---- Prologue woven with the it=0 j-loops, unit-interleaved ----
            def weave(units, jts):
                q = list(jts)
                for i, u in enumerate(units):
                    u()
                    if i < len(units) - 1 and q:
                        it_, p_, po_, jt_ = q.pop(0)
                        emit_jt(it_, p_, po_, jt_)
                for it_, p_, po_, jt_ in q:
                    emit_jt(it_, p_, po_, jt_)

            prologue_group(0)
            qt_proj(0, 0)
            qt_proj(1, 0)
            a0 = alloc_po(0, 0, 0)
            weave(
                [lambda t=t: transpose_unit(4 + t) for t in range(4)]
                + [lambda: kv_unit(1), lambda: v_unit(1)],
                [(0, 0, a0, jt) for jt in range(0, 4)],
            )
            weave(
                [lambda t=t: transpose_unit(8 + t) for t in range(4)]
                + [lambda: kv_unit(2), lambda: v_unit(2)],
                [(0, 0, a0, jt) for jt in range(4, 8)],
            )
            ah = alloc_oh(0, 0)
            flush_half(a0, ah)
            qt_proj(0, 1)
            b0 = alloc_po(0, 1, 0)
            weave(
                [lambda t=t: transpose_unit(12 + t) for t in range(4)]
                + [lambda: kv_unit(3), lambda: v_unit(3)],
                [(0, 1, b0, jt) for jt in range(0, 4)],
            )
            for jt in range(4, 8):
                emit_jt(0, 1, b0, jt)
            bh = alloc_oh(0, 1)
            flush_half(b0, bh)
            qt_proj(1, 1)
            a1 = alloc_po(0, 0, 1)
            for jt in range(8, 16):
                emit_jt(0, 0, a1, jt)
            emit_normalize(0, 0, ah, a1, defer=True)
            b1 = alloc_po(0, 1, 1)
            for jt in range(8, 16):
                emit_jt(0, 1, b1, jt)
            emit_normalize(0, 1, bh, b1, defer=True)
            queue_outproj(0)

            # ---- Remaining (it, p) j-loops ----
            seq = [(it, p) for it in range(1, IT) for p in range(PAIRS)]
            for it, p in seq:
                po = alloc_po(it, p, 0)
                for jt in range(0, 8):
                    emit_jt(it, p, po, jt)
                oh = alloc_oh(it, p)
                flush_half(po, oh)
                po2 = alloc_po(it, p, 1)
                for jt in range(8, 16):
                    emit_jt(it, p, po2, jt)
                    if jt == 12 and it + 1 < IT:
                        qt_proj(p, it + 1)
                emit_normalize(it, p, oh, po2, defer=(it, p) != (IT - 1, PAIRS - 1))
                if p == PAIRS - 1:
                    queue_outproj(it)
            for fn in pending:
                fn()
            del pending[:]

    nc.compile()
    return nc


_NC = None


def _get_nc():
    global _NC
    if _NC is None:
        _NC = _build()
    return _NC


def _prep_in_maps(x, Wq, Wkv, Wout):
    in_maps = []
    wk = Wkv[:, 0:DIM_HEAD]
    wv = Wkv[:, DIM_HEAD:]
    wkv_packed = np.ascontiguousarray(
        np.concatenate([wk, wk, wv], axis=1), dtype=np.float32
    )
    for core in range(N_CORES):
        b, g = divmod(core, 4)
        h0 = g * HEADS_PER_CORE
        wq = np.ascontiguousarray(
            np.transpose(Wq[h0:h0 + HEADS_PER_CORE], (1, 0, 2)).reshape(DIM, INNER),
            dtype=np.float32,
        )
        wout = np.ascontiguousarray(
            Wout[h0 * DIM_HEAD:(h0 + HEADS_PER_CORE) * DIM_HEAD], dtype=np.float32
        )
        in_maps.append(
            {
                "x": np.ascontiguousarray(x[b], dtype=np.float32),
                "wq": wq,
                "wkv": wkv_packed,
                "wout": wout,
            }
        )
    return in_maps


def _ensure_hook_shim():
    """bass_utils imports antenv.axon_hooks when tracing is requested via
    env (BASS_TRACE); that module is absent on this image. Provide a no-op
    fallback so an inherited env var cannot break a plain run."""
    try:
        import antenv.axon_hooks  # noqa: F401
    except Exception:
        import sys
        import types

        m = types.ModuleType("antenv.axon_hooks")
        m.get_axon_ntff_profile_hook = lambda: None
        m.set_axon_ntff_profile_hook = lambda h: None
        sys.modules["antenv.axon_hooks"] = m


def run(inputs, trace=False):
    """Run on 8 cores; returns (full_output, BassKernelResults)."""
    _ensure_hook_shim()
    nc = _get_nc()
    in_maps = _prep_in_maps(
        np.asarray(inputs["x"]),
        np.asarray(inputs["Wq"]),
        np.asarray(inputs["Wkv"]),
        np.asarray(inputs["Wout"]),
    )
    res = run_bass_kernel_spmd(
        nc, in_maps, core_ids=list(range(N_CORES)), trace=trace
    )
    out = np.zeros((B, N, DIM), dtype=np.float32)
    for core in range(N_CORES):
        b = core // 4
        out[b] += res.results[core]["out"]
    return out, res


def kernel(**inputs) -> np.ndarray:
    out, _ = run(inputs, trace=bool(os.environ.get("BASS_KERNEL_TRACE")))
    return out
